# revision 36
# baseline (speedup 1.0000x reference)
"""Trainium2 Bass kernel for nn_Averager (pooling, 3-level box-average).

Math (verified vs reference): per sample, with input x[n, i, c] where
n = (n5 n4 n3 n2 n1 n0) base-4 digits, c = (c2 c1 c0) base-4 digits:
  out[:, :, 0, :] = x[:, :, 0, :]
  out1[n, c] = E[n4, n2, c2, c0, n0, c1],
      E[r5, r4, r3, r0; g2, g1] = mean over (n2, n1, c0) of x1
  out2[n, c] = G[c2, c1, c0],
      G[p, q, r] = mean over (n4, n3, n1, n0, c1, c0) of x2 with
      (n5, c2in, n2) = (p, q, r)

Sharding: data-parallel over batch, 4 samples per core on 8 cores,
processed as 2 groups of 2 samples.

Layout (pair-contiguous): SBUF partition p = b*64 + n//64 =
(b, n5, n4, n3); free j = n % 64 = 16*n2 + 4*n1 + n0, row (i, c).
A 6MB group is contiguous in DRAM and per-partition contiguous in SBUF.

DMA plan (everything is large and fully contiguous — the output is
assembled COMPLETELY in SBUF, including replicating the single L2 row
into all 64 (j, i=2) rows, so the store is a straight 2-D stream; the
baseline's per-level region stores produced 256B-chunk strided writes
that ran at ~half HBM rate):
  - in:  2 x 3MiB per group on SWDGE (nc.gpsimd), FIFO so group 0
    lands first and compute starts after the first half.  Measured
    best: moving any input onto a HWDGE ring cost 3-9us (SDMA engine
    15 paces ~8% slower on HWDGE-fed input streams here).
  - out: 2 x 3MiB contiguous per group on the ACT HWDGE ring
    (nc.scalar), issued by the ACT engine itself so program order
    covers the evac/fill writes.
  - the factored selector constants (71KB bf16) ride the SP HWDGE
    ring (needed by the first matmuls at ~27us, landing ~10us).
  - SDMA engine 15 (~21 vs ~27 GB/s on this part, serving partitions
    92-95/124-127 per port=bits[4:2]<<1|bit[6]) carries 1/16 of every
    128-partition transfer and is the true bottleneck: kernel time =~
    swdge_start + eng15 in-bytes + eng15 out-bytes + drain tail.
    Partition-range splitting to route around it is PESSIMAL
    (non-128-partition DMAs get a poor descriptor fan-out, ~100GB/s),
    so the schedule instead keeps engine 15 saturated end-to-end: the
    first out-DMA must issue BEFORE the last in-byte lands.

Compute: the engine pipeline is DVE -> PE -> ACT.  Stage-A reductions
are lane-local on DVE (reduced digits live in the free dim), split by
input half (u0/t4a then u1/t4b) to start right after each in-DMA
lands.  The per-block selectors factor as S1_blk = diag(M[:,blk]) @ B
with a SHARED routing matrix B (see _make_s12), so DVE block-masks
A/A2 with one broadcast-multiply each and the PE does just 3 wide
bf16 matmuls (PE fp32 is 1/4 rate; selector values 1/64, 1/4096 are
exact in bf16 and A/A2 round at ~2^-9 relative — far inside the 2e-2
gate).  The PSUM evacs
and L2 row fills into the output tile run on the otherwise-idle ACT
engine, so group g's evac/store phase overlaps group g+1's DVE
stage-A — with everything on DVE those two serialized (engines are
in-order) and the whole kernel became DVE-limited.

Single-sem-wait discipline: out-DMAs are issued BY the ACT engine, so
ACT program order covers every compute write into the output tile and
the one explicit wait is the in-DMA semaphore (for the untouched L0
bytes).  Matmuls read the bf16 selector tile straight from its HWDGE
load (one sem) and A/A2 from DVE (one sem).
"""

import numpy as np

N_CORES = 8
B_FULL = 32
B_CORE = B_FULL // N_CORES  # 4
N = 4096
LVL = 3
C = 64


def _make_s12():
    """Factored routing selectors, pair layout k = 64*b + 16*k5 + 4*k4 + k3.

    The original per-block selectors factor through shared routing
    matrices:
        S1[k, blk=(n2o,c2o), m] = B[k, m] * M[k, blk]
            B[k, m] = 1/64   iff b(k)==b(m), k5(k)==m4(m)
            M[k, blk] = 1    iff k4(k)==n2o, k3(k)==c2o
        S2[k, c2o, m] = C[k, m] * M2[k, c2o]
            C[k, m] = 1/4096 iff b(k)==b(m)
            M2[k, c2o] = 1   iff k5(k)==c2o
    so the kernel ships only [B | C | M | M2] = [128, 276] bf16 (71KB
    instead of 640KB) and masks A/A2 in the free dim instead of using
    16 distinct stationary operands.
    """
    import ml_dtypes

    k = np.arange(128)
    b, k5, k4, k3 = k >> 6, (k >> 4) & 3, (k >> 2) & 3, k & 3
    m = np.arange(128)
    bm, m4 = m >> 6, (m >> 2) & 3
    B = ((b[:, None] == bm[None, :]) & (k5[:, None] == m4[None, :])
         ).astype(np.float32) / 64.0
    C = (b[:, None] == bm[None, :]).astype(np.float32) / 4096.0
    blk = np.arange(16)
    M = ((k4[:, None] == (blk[None, :] >> 2)) & (k3[:, None] == (blk[None, :] & 3))
         ).astype(np.float32)
    M2 = (k5[:, None] == np.arange(4)[None, :]).astype(np.float32)
    return np.ascontiguousarray(
        np.concatenate([B, C, M, M2], axis=1).astype(ml_dtypes.bfloat16)
    )


def _build_nc():
    from contextlib import nullcontext

    import concourse.bass as bass
    import concourse.tile as tile
    from concourse import mybir

    dt = mybir.dt.float32
    bf = mybir.dt.bfloat16
    X = mybir.AxisListType.X
    ADD = mybir.AluOpType.add

    from concourse import bacc
    nc = bacc.Bacc()
    x = nc.declare_dram_parameter("x", [B_CORE, N, LVL, C], dt, isOutput=False)
    s12 = nc.declare_dram_parameter("s12", [128, 276], bf, isOutput=False)
    out = nc.declare_dram_parameter("out", [B_CORE, N, LVL, C], dt, isOutput=True)

    with tile.TileContext(nc) as tc:
        with (
            tc.tile_pool(name="consts", bufs=1) as cpool,
            tc.tile_pool(name="xin", bufs=2) as xpool,
            tc.tile_pool(name="tmp", bufs=1) as tpool,
            tc.tile_pool(name="psum", bufs=2, space="PSUM") as ppool,
        ):
            # const leads the SWDGE FIFO (71KB =~ 0.3us ahead of the
            # ins); keeps the whole kernel strictly single-queue-phased
            # (concurrent DMA queues measurably degrade aggregate rate)
            s12sb = cpool.tile([128, 276], bf, tag="s12")
            nc.gpsimd.dma_start(s12sb[:], s12[:])
            Bsb = s12sb[:, 0:128]
            Csb = s12sb[:, 128:256]
            Msb = s12sb[:, 256:272]
            M2sb = s12sb[:, 272:276]

            for g in range(B_CORE // 2):
                bs = slice(2 * g, 2 * g + 2)
                xt = xpool.tile([128, 12288], dt, tag="xt")
                # split the 6MB load so compute starts after the first
                # half (j<32) lands.  All input halves queue FIFO on
                # SWDGE — measured best; moving any of them to a HWDGE
                # ring (v6/v7 experiments) cost 3-9us.
                xsrc = x[bs].rearrange("b (ph j) i c -> (b ph) (j i c)", ph=64)
                nc.gpsimd.dma_start(xt[:, 0:6144], xsrc[:, 0:6144])
                nc.gpsimd.dma_start(xt[:, 6144:12288], xsrc[:, 6144:12288])
                # Model-time floor for group 1's compute: the list
                # scheduler otherwise slots group 1's (blocked) ops
                # ahead of group 0's ready ops in the same in-order
                # engine streams, stalling ACT for ~10us and starving
                # the out ring.  Placement-only: runtime waits are
                # still the dep semaphores.
                gctx = tc.tile_wait_until(0.4) if g else nullcontext()
                gctx.__enter__()
                xtv = xt[:].rearrange(
                    "p (j i c) -> p j i c", j=64, i=3, c=64
                )

                v = xt[:].rearrange(
                    "p (n2 n1 n0 i c) -> p n2 n1 n0 i c",
                    n2=4, n1=4, n0=4, i=3, c=64,
                )
                xw = xt[:].rearrange(
                    "p (j i c2 cc) -> p j i c2 cc", j=64, i=3, c2=4, cc=16
                )
                t4 = tpool.tile([128, 256], dt, tag="t4")
                t4v = t4[:].rearrange("p (j c2) -> p j c2", j=64, c2=4)

                # ---- stage A on the first input half (j < 32) ----
                u0 = tpool.tile([128, 1024], dt, tag="u0")
                nc.vector.tensor_add(
                    u0[:].rearrange("p (n1 n0 c) -> p n1 n0 c", n1=4, n0=4, c=64),
                    v[:, 0, :, :, 1, :], v[:, 1, :, :, 1, :],
                )
                nc.vector.tensor_reduce(
                    t4v[:, 0:32, :], xw[:, 0:32, 2, :, :], axis=X, op=ADD,
                )
                # ---- stage A on the second input half ----
                u1 = tpool.tile([128, 1024], dt, tag="u1")
                nc.vector.tensor_add(
                    u1[:].rearrange("p (n1 n0 c) -> p n1 n0 c", n1=4, n0=4, c=64),
                    v[:, 2, :, :, 1, :], v[:, 3, :, :, 1, :],
                )
                # ---- L1 stage A tail: lane-local sum over (n2, n1, c0) ----
                # (emitted before t4b: the L1 path feeds the first-half
                # evac chain, the critical out-issue path)
                w = tpool.tile([128, 1024], dt, tag="w")
                nc.vector.tensor_add(w[:], u0[:], u1[:])
                h1 = tpool.tile([128, 512], dt, tag="h1")
                nc.vector.tensor_add(h1[:], w[:, 0:512], w[:, 512:1024])
                h2 = tpool.tile([128, 256], dt, tag="h2")
                nc.vector.tensor_add(h2[:], h1[:, 0:256], h1[:, 256:512])
                # reduce c0, write A with free = 16*c2 + 4*c1 + n0
                Af = tpool.tile([128, 64], dt, tag="Af")
                nc.vector.tensor_reduce(
                    Af[:].rearrange("p (c2 c1 n0) -> p n0 c2 c1", c2=4, c1=4, n0=4),
                    h2[:].rearrange(
                        "p (n0 c2 c1 c0) -> p n0 c2 c1 c0", n0=4, c2=4, c1=4, c0=4
                    ),
                    axis=X, op=ADD,
                )
                A = tpool.tile([128, 64], bf, tag="A")
                nc.vector.tensor_copy(A[:], Af[:])
                # mask A into the 16 (n2o, c2o) routing blocks, split
                # by half so c1p's first matmul starts one op earlier:
                # Awide[k, blk*64+t] = M[k, blk] * A[k, t]
                Aw = tpool.tile([128, 1024], bf, tag="Aw")
                Awv = Aw[:].rearrange("p (blk t) -> p blk t", blk=16, t=64)
                Mv = Msb.rearrange("p (blk one) -> p blk one", blk=16, one=1)
                Av = A[:].rearrange("p (one t) -> p one t", one=1, t=64)
                nc.vector.tensor_mul(
                    Awv[:, 0:8, :],
                    Mv[:, 0:8, :].broadcast_to((128, 8, 64)),
                    Av.broadcast_to((128, 8, 64)),
                )
                nc.vector.tensor_mul(
                    Awv[:, 8:16, :],
                    Mv[:, 8:16, :].broadcast_to((128, 8, 64)),
                    Av.broadcast_to((128, 8, 64)),
                )
                # ---- L2 stage A tail (t4b deferred to run AFTER the
                # L1 path: the L1 chain feeds the first out-DMA, and
                # the scheduler otherwise runs t4b first, pushing the
                # out-issue ~2.3us; floor pins the placement) ----
                with tc.tile_wait_until(0.1 + g * 0.4):
                    nc.vector.tensor_reduce(
                        t4v[:, 32:64, :], xw[:, 32:64, 2, :, :],
                        axis=X, op=ADD,
                    )
                    A2f = tpool.tile([128, 16], dt, tag="A2f")
                    nc.vector.tensor_reduce(
                        A2f[:].rearrange("p (c2 n2) -> p n2 c2", c2=4, n2=4),
                        t4[:].rearrange(
                            "p (n2 nn c2) -> p n2 c2 nn", n2=4, nn=16, c2=4
                        ),
                        axis=X, op=ADD,
                    )
                    A2 = tpool.tile([128, 16], bf, tag="A2")
                    nc.vector.tensor_copy(A2[:], A2f[:])
                    # A2wide[k, c2o*16+t] = M2[k, c2o] * A2[k, t]
                    A2w = tpool.tile([128, 64], bf, tag="A2w")
                    nc.vector.tensor_mul(
                        A2w[:].rearrange("p (c2o t) -> p c2o t", c2o=4, t=16),
                        M2sb.rearrange(
                            "p (c2o one) -> p c2o one", c2o=4, one=1
                        ).broadcast_to((128, 4, 16)),
                        A2[:].rearrange("p (one t) -> p one t", one=1, t=16)
                        .broadcast_to((128, 4, 16)),
                    )

                # ---- PE: 3 wide bf16 matmuls with shared stationaries
                # c1p free = 64*(4*n2o + c2o) + (16*n0o + 4*c1o + c0o);
                # one 512-wide matmul per PSUM bank (first = blks of
                # n2o 0,1 so the first-half evac chain starts early);
                # gp free = 16*c2o + (4*c1o + c0o)
                c1p = ppool.tile([128, 1024], dt, tag="c1p")
                gp = ppool.tile([128, 64], dt, tag="gp")
                nc.tensor.matmul(
                    c1p[:, 0:512], Bsb, Aw[:, 0:512], start=True, stop=True,
                )
                nc.tensor.matmul(
                    gp[:, 0:64], Csb, A2w[:, 0:64], start=True, stop=True,
                )
                nc.tensor.matmul(
                    c1p[:, 512:1024], Bsb, Aw[:, 512:1024],
                    start=True, stop=True,
                )

                # ---- evac + L2 fill, first output half (j < 32) ----
                c1e = c1p[:].rearrange(
                    "p (n2o c2o n0 cc) -> p n2o c2o n0 cc",
                    n2o=4, c2o=4, n0=4, cc=16,
                )
                xts = xt[:].rearrange(
                    "p (n2 n1 n0 i c2 cc) -> p n2 n1 c2 n0 i cc",
                    n2=4, n1=4, n0=4, i=3, c2=4, cc=16,
                )
                gpb = gp[:].rearrange("p (j c) -> p j c", j=1, c=64)
                for n2o in range(2):
                    for n1o in range(4):
                        nc.scalar.copy(
                            xts[:, n2o, n1o, :, :, 1, :],
                            c1e[:, n2o, :, :, :],
                        )
                nc.scalar.copy(
                    xtv[:, 0:32, 2, :], gpb.broadcast_to((128, 32, 64))
                )
                outv = out[bs].rearrange("b (ph j) i c -> (b ph) (j i c)", ph=64)
                nc.scalar.dma_start(outv[:, 0:6144], xt[:, 0:6144])

                # ---- evac + L2 fill, second output half ----
                # model-time floor so the scheduler never places this
                # (later-ready) chain ahead of the first half's on ACT
                with tc.tile_wait_until(0.2 + g * 0.4):
                    for n2o in range(2, 4):
                        for n1o in range(4):
                            nc.scalar.copy(
                                xts[:, n2o, n1o, :, :, 1, :],
                                c1e[:, n2o, :, :, :],
                            )
                    nc.scalar.copy(
                        xtv[:, 32:64, 2, :], gpb.broadcast_to((128, 32, 64))
                    )
                    nc.scalar.dma_start(outv[:, 6144:12288], xt[:, 6144:12288])
                gctx.__exit__(None, None, None)
    nc.compile()
    return nc


_NC_CACHE = {}


def _get_nc():
    if "nc" not in _NC_CACHE:
        _NC_CACHE["nc"] = _build_nc()
    return _NC_CACHE["nc"]


def kernel(**inputs: np.ndarray) -> np.ndarray:
    from concourse.bass_utils import run_bass_kernel_spmd

    x = np.ascontiguousarray(inputs["x"], dtype=np.float32)
    assert x.shape == (B_FULL, N, LVL, C), x.shape
    S12 = _make_s12()
    nc = _get_nc()
    in_maps = [
        {"x": np.ascontiguousarray(x[k * B_CORE:(k + 1) * B_CORE]),
         "s12": S12}
        for k in range(N_CORES)
    ]
    res = run_bass_kernel_spmd(nc, in_maps, list(range(N_CORES)))
    outs = [res.results[k]["out"] for k in range(N_CORES)]
    return np.ascontiguousarray(np.concatenate(outs, axis=0))
